# revision 2
# baseline (speedup 1.0000x reference)
"""Multi-headed causal self-attention on 8 Trainium2 NeuronCores.

Sharding: tensor-parallel over heads (2 of 16 heads per core).
Each core computes Q/K/V projections for its 256-wide feature slice,
causal attention for its 2 heads, and the partial output projection
through its slice of Wo.  The host sums the 8 partial outputs and adds
the analytically-folded constant row  bo + Wo @ bv  (softmax rows sum
to one, so V's bias contributes a constant vector through Wo).

On-chip layout (all matmuls in float32r = full PE rate):
  - X is passed host-pre-transposed as XT [D, B*S] so the contraction
    dim of every projection lands on SBUF partitions.
  - Q, K are produced feature-major [d_head, tok]; V token-major
    [tok, d_head].
  - scores are computed transposed (scoresT [k, q]) so that exp@V needs
    no transposes and softmax denominators come from a ones-matmul.
  - softmax skips max-subtraction (scores are bounded ~|5| for these
    input scales); causal masking adds -60 to invalid logits of
    diagonal 128x128 blocks before exp, off-diagonal blocks above the
    diagonal are simply never computed.
  - the per-token denominators are applied after the out-projection
    (per head), where they sit on the partition axis.
"""

import numpy as np

import concourse.bass as bass  # noqa: F401  (registers engine types)
import concourse.tile as tile
from concourse import bacc, mybir
from concourse.bass_utils import run_bass_kernel_spmd

N_CORES = 8
B, S, D = 2, 2048, 2048
H, DH = 16, 128
HPC = H // N_CORES          # heads per core
DSH = HPC * DH              # per-core feature slice width (256)
TOK = B * S
P = 128                     # SBUF partitions
QCW = 512                   # query-chunk width (matmul moving dim)
NQC = S // QCW              # q-chunks per batch
NKT = S // P                # k-tiles per batch
KTPC = QCW // P             # k-tiles per q-chunk
NJC = D // QCW              # output column chunks
SCALE = float(1.0 / np.sqrt(np.sqrt(DH)))
MASK_NEG = -60.0

F32 = mybir.dt.float32
F32R = mybir.dt.float32r
AFT = mybir.ActivationFunctionType

TRACE = False
LAST = {}

_nc = None


def _emit(tc, t):
    from contextlib import ExitStack

    nc = tc.nc
    with ExitStack() as ctx:
        const = ctx.enter_context(tc.tile_pool(name="const", bufs=1))
        xtp = ctx.enter_context(tc.tile_pool(name="xtp", bufs=3))
        kvp = ctx.enter_context(tc.tile_pool(name="kvp", bufs=1))
        qch = ctx.enter_context(tc.tile_pool(name="qch", bufs=2))
        ach = ctx.enter_context(tc.tile_pool(name="ach", bufs=2))
        expp = ctx.enter_context(tc.tile_pool(name="expp", bufs=4))
        sacp = ctx.enter_context(tc.tile_pool(name="sacp", bufs=2))
        mscp = ctx.enter_context(tc.tile_pool(name="mscp", bufs=3))
        smlp = ctx.enter_context(tc.tile_pool(name="smlp", bufs=2))
        outsp = ctx.enter_context(tc.tile_pool(name="outsp", bufs=4))
        psA = ctx.enter_context(tc.tile_pool(name="psA", bufs=3, space="PSUM"))
        psS = ctx.enter_context(tc.tile_pool(name="psS", bufs=2, space="PSUM"))
        psT = ctx.enter_context(tc.tile_pool(name="psT", bufs=2, space="PSUM"))
        psD = ctx.enter_context(tc.tile_pool(name="psD", bufs=1, space="PSUM"))

        # ---- constants ----
        wq_sb = const.tile([P, NKT, DSH], F32R)
        nc.sync.dma_start(out=wq_sb, in_=t["wqt"].rearrange("(k p) m -> p k m", p=P))
        wk_sb = const.tile([P, NKT, DSH], F32R)
        nc.sync.dma_start(out=wk_sb, in_=t["wkt"].rearrange("(k p) m -> p k m", p=P))
        wv_sb = const.tile([P, NKT, DSH], F32R)
        nc.sync.dma_start(out=wv_sb, in_=t["wvt"].rearrange("(k p) m -> p k m", p=P))
        wo_sb = const.tile([P, HPC, D], F32R)
        nc.sync.dma_start(out=wo_sb, in_=t["wot"].rearrange("(h p) n -> p h n", p=P))
        tri_sb = const.tile([P, P], F32)
        nc.sync.dma_start(out=tri_sb, in_=t["tri"])
        bq_sb = const.tile([P, HPC, 1], F32)
        nc.sync.dma_start(out=bq_sb, in_=t["bqs"].rearrange("h p o -> p h o"))
        bk_sb = const.tile([P, HPC, 1], F32)
        nc.sync.dma_start(out=bk_sb, in_=t["bks"].rearrange("h p o -> p h o"))
        ones2 = const.tile([P, 2], F32R)
        nc.sync.dma_start(out=ones2, in_=t["onesc"])

        xt_view = t["xt"].rearrange("(k p) (c q) -> c p k q", p=P, q=QCW)

        for b in range(B):
            k_sb = kvp.tile([P, HPC, S], F32R, tag="k")
            v_sb = kvp.tile([P, NKT, DSH], F32R, tag="v")
            for qc in range(NQC):
                c = b * NQC + qc

                # ---- QKV projections for token chunk c ----
                xt0 = xtp.tile([P, NKT // 2, QCW], F32R, tag="xt")
                xt1 = xtp.tile([P, NKT // 2, QCW], F32R, tag="xt")
                nc.sync.dma_start(out=xt0, in_=xt_view[c][:, 0:8, :])
                nc.sync.dma_start(out=xt1, in_=xt_view[c][:, 8:16, :])
                xth = (xt0, xt1)
                q_sb = qch.tile([P, HPC, QCW], F32R, tag="q")
                for j in range(HPC):
                    qp = psA.tile([P, QCW], F32, tag="ps")
                    for k in range(NKT):
                        nc.tensor.matmul(
                            qp, wq_sb[:, k, j * DH:(j + 1) * DH],
                            xth[k // 8][:, k % 8, :],
                            start=(k == 0), stop=(k == NKT - 1))
                    nc.scalar.activation(q_sb[:, j, :], qp, AFT.Identity,
                                         bias=bq_sb[:, j, :], scale=SCALE)
                    kp = psA.tile([P, QCW], F32, tag="ps")
                    for k in range(NKT):
                        nc.tensor.matmul(
                            kp, wk_sb[:, k, j * DH:(j + 1) * DH],
                            xth[k // 8][:, k % 8, :],
                            start=(k == 0), stop=(k == NKT - 1))
                    nc.scalar.activation(k_sb[:, j, qc * QCW:(qc + 1) * QCW], kp,
                                         AFT.Identity, bias=bk_sb[:, j, :], scale=SCALE)
                for tt in range(KTPC):
                    vp = psA.tile([P, QCW], F32, tag="ps")
                    for k in range(NKT):
                        nc.tensor.matmul(
                            vp[:, 0:DSH],
                            xth[k // 8][:, k % 8, tt * P:(tt + 1) * P],
                            wv_sb[:, k, :],
                            start=(k == 0), stop=(k == NKT - 1))
                    nc.scalar.copy(v_sb[:, qc * KTPC + tt, :], vp[:, 0:DSH])

                # ---- causal attention for q-chunk qc, both heads ----
                a_sb = ach.tile([P, HPC, QCW], F32R, tag="a")
                rc_tiles = {}
                nkt_q = (qc + 1) * KTPC
                for h in range(HPC):
                    sacc = sacp.tile([P, QCW], F32, tag="sacc")
                    at = psT.tile([P, QCW], F32, tag="at")
                    for kt in range(nkt_q):
                        tdiag = kt - qc * KTPC
                        off = max(tdiag, 0) * P
                        w = QCW - off
                        sp = psS.tile([P, QCW], F32, tag="sc")
                        nc.tensor.matmul(
                            sp[:, 0:w], k_sb[:, h, kt * P:(kt + 1) * P],
                            q_sb[:, h, off:QCW], start=True, stop=True)
                        et = expp.tile([P, QCW], F32R, tag="exp")
                        if tdiag >= 0:
                            msc = mscp.tile([P, P], F32, tag="msc")
                            nc.vector.tensor_add(msc, sp[:, 0:P], tri_sb)
                            nc.scalar.activation(et[:, 0:P], msc, AFT.Exp)
                            if w > P:
                                nc.scalar.activation(et[:, P:w], sp[:, P:w], AFT.Exp)
                        else:
                            nc.scalar.activation(et[:, 0:w], sp[:, 0:w], AFT.Exp)
                        etf = et.bitcast(F32)
                        if kt == 0:
                            nc.vector.tensor_copy(sacc, etf)
                        else:
                            nc.vector.tensor_add(sacc[:, off:QCW], sacc[:, off:QCW],
                                                 etf[:, 0:w])
                        nc.tensor.matmul(
                            at[:, off:QCW], v_sb[:, kt, h * DH:(h + 1) * DH],
                            et[:, 0:w], start=(kt == 0), stop=(kt == nkt_q - 1))
                    nc.scalar.copy(a_sb[:, h, :], at)
                    sacc_r = smlp.tile([P, QCW], F32R, tag="saccr", bufs=2)
                    nc.scalar.copy(sacc_r, sacc)
                    dn = psD.tile([P, 2 * KTPC], F32, tag="dn")
                    for tt in range(KTPC):
                        nc.tensor.matmul(dn[:, 2 * tt:2 * tt + 2],
                                         sacc_r[:, tt * P:(tt + 1) * P], ones2,
                                         start=True, stop=True)
                    rc = smlp.tile([P, 2 * KTPC], F32, tag="rc", bufs=4)
                    nc.vector.reciprocal(rc, dn)
                    rc_tiles[h] = rc

                # ---- partial out-projection for this chunk's tokens ----
                for tt in range(KTPC):
                    r0 = rc_tiles[0][:, 2 * tt:2 * tt + 1]
                    r1 = rc_tiles[1][:, 2 * tt:2 * tt + 1]
                    for jc in range(NJC):
                        p0 = psA.tile([P, QCW], F32, tag="ps")
                        nc.tensor.matmul(p0, a_sb[:, 0, tt * P:(tt + 1) * P],
                                         wo_sb[:, 0, jc * QCW:(jc + 1) * QCW],
                                         start=True, stop=True)
                        p1 = psA.tile([P, QCW], F32, tag="ps")
                        nc.tensor.matmul(p1, a_sb[:, 1, tt * P:(tt + 1) * P],
                                         wo_sb[:, 1, jc * QCW:(jc + 1) * QCW],
                                         start=True, stop=True)
                        tmp = outsp.tile([P, QCW], F32, tag="ot")
                        nc.vector.tensor_scalar_mul(tmp, p0, r0)
                        ot = outsp.tile([P, QCW], F32, tag="ot")
                        nc.vector.scalar_tensor_tensor(
                            ot, p1, r1, tmp,
                            mybir.AluOpType.mult, mybir.AluOpType.add)
                        row0 = b * S + (qc * KTPC + tt) * P
                        nc.sync.dma_start(
                            out=t["outp"][row0:row0 + P, jc * QCW:(jc + 1) * QCW],
                            in_=ot)


def _build():
    nc = bacc.Bacc("TRN2", target_bir_lowering=False, debug=False,
                   num_devices=N_CORES)
    t = {
        "xt": nc.dram_tensor("xt", [D, TOK], F32R, kind="ExternalInput").ap(),
        "wqt": nc.dram_tensor("wqt", [D, DSH], F32R, kind="ExternalInput").ap(),
        "wkt": nc.dram_tensor("wkt", [D, DSH], F32R, kind="ExternalInput").ap(),
        "wvt": nc.dram_tensor("wvt", [D, DSH], F32R, kind="ExternalInput").ap(),
        "wot": nc.dram_tensor("wot", [DSH, D], F32R, kind="ExternalInput").ap(),
        "bqs": nc.dram_tensor("bqs", [HPC, P, 1], F32, kind="ExternalInput").ap(),
        "bks": nc.dram_tensor("bks", [HPC, P, 1], F32, kind="ExternalInput").ap(),
        "tri": nc.dram_tensor("tri", [P, P], F32, kind="ExternalInput").ap(),
        "onesc": nc.dram_tensor("onesc", [P, 2], F32R, kind="ExternalInput").ap(),
        "outp": nc.dram_tensor("outp", [TOK, D], F32, kind="ExternalOutput").ap(),
    }
    with tile.TileContext(nc) as tc:
        _emit(tc, t)
    nc.compile()
    return nc


def _program():
    global _nc
    if _nc is None:
        _nc = _build()
    return _nc


def kernel(X, Wq, bq, Wk, bk, Wv, bv, Wo, bo):
    X = np.asarray(X, np.float32)
    Wq = np.asarray(Wq, np.float32)
    Wk = np.asarray(Wk, np.float32)
    Wv = np.asarray(Wv, np.float32)
    Wo = np.asarray(Wo, np.float32)
    bq = np.asarray(bq, np.float32)
    bk = np.asarray(bk, np.float32)
    bv = np.asarray(bv, np.float32)
    bo = np.asarray(bo, np.float32)

    nc = _program()

    XT = np.ascontiguousarray(X.reshape(TOK, D).T)
    tri = np.where(np.arange(P)[:, None] <= np.arange(P)[None, :],
                   np.float32(0.0), np.float32(MASK_NEG)).astype(np.float32)
    ones_col = np.ones((P, 2), np.float32)

    in_maps = []
    for c in range(N_CORES):
        J = slice(c * DSH, (c + 1) * DSH)
        in_maps.append({
            "xt": XT,
            "wqt": np.ascontiguousarray(Wq[J, :].T),
            "wkt": np.ascontiguousarray(Wk[J, :].T),
            "wvt": np.ascontiguousarray(Wv[J, :].T),
            "wot": np.ascontiguousarray(Wo[:, J].T),
            "bqs": (bq[J] * SCALE).reshape(HPC, P, 1).astype(np.float32),
            "bks": (bk[J] * SCALE).reshape(HPC, P, 1).astype(np.float32),
            "tri": tri,
            "onesc": ones_col,
        })

    res = run_bass_kernel_spmd(nc, in_maps, list(range(N_CORES)), trace=TRACE)
    LAST["res"] = res

    out = res.results[0]["outp"].copy()
    for c in range(1, N_CORES):
        out += res.results[c]["outp"]
    out += (bo + Wo @ bv)[None, :].astype(np.float32)
    return out.reshape(B, S, D).astype(np.float32)


# revision 3
# speedup vs baseline: 1.1042x; 1.1042x over previous
"""Multi-headed causal self-attention on 8 Trainium2 NeuronCores.

Sharding: tensor-parallel over heads (2 of 16 heads per core).
Each core computes Q/K/V projections for its 256-wide feature slice,
causal attention for its 2 heads, and the partial output projection
through its slice of Wo.  The host sums the 8 partial outputs and adds
the analytically-folded constant row  bo + Wo @ bv  (softmax rows sum
to one, so V's bias contributes a constant vector through Wo).

On-chip layout (all matmuls in float32r = full PE rate):
  - X is passed host-pre-transposed as XT [D, B*S] so the contraction
    dim of every projection lands on SBUF partitions.
  - Q, K are produced feature-major [d_head, tok]; V token-major
    [tok, d_head].
  - scores are computed transposed (scoresT [k, q]) so that exp@V needs
    no transposes and softmax denominators come from a ones-matmul.
  - softmax skips max-subtraction (scores are bounded ~|5| for these
    input scales); causal masking adds -60 to invalid logits of
    diagonal 128x128 blocks before exp, off-diagonal blocks above the
    diagonal are simply never computed.
  - the per-token denominators are applied after the out-projection
    (per head), where they sit on the partition axis.
"""

import ml_dtypes
import numpy as np

import concourse.bass as bass  # noqa: F401  (registers engine types)
import concourse.tile as tile
from concourse import bacc, mybir
from concourse.bass_utils import run_bass_kernel_spmd

N_CORES = 8
B, S, D = 2, 2048, 2048
H, DH = 16, 128
HPC = H // N_CORES          # heads per core
DSH = HPC * DH              # per-core feature slice width (256)
TOK = B * S
P = 128                     # SBUF partitions
QCW = 512                   # query-chunk width (matmul moving dim)
NQC = S // QCW              # q-chunks per batch
NKT = S // P                # k-tiles per batch
KTPC = QCW // P             # k-tiles per q-chunk
NJC = D // QCW              # output column chunks
SCALE = float(1.0 / np.sqrt(np.sqrt(DH)))
MASK_NEG = -60.0

F32 = mybir.dt.float32
F32R = mybir.dt.float32r
BF16 = mybir.dt.bfloat16
MMD = BF16                  # matmul operand dtype (bf16 -> FWL weight loads)
AFT = mybir.ActivationFunctionType

TRACE = False
LAST = {}

_nc = None


def _emit(tc, t):
    from contextlib import ExitStack

    nc = tc.nc
    with ExitStack() as ctx:
        const = ctx.enter_context(tc.tile_pool(name="const", bufs=1))
        xtp = ctx.enter_context(tc.tile_pool(name="xtp", bufs=4))
        kvp = ctx.enter_context(tc.tile_pool(name="kvp", bufs=2))
        qch = ctx.enter_context(tc.tile_pool(name="qch", bufs=3))
        ach = ctx.enter_context(tc.tile_pool(name="ach", bufs=3))
        expp = ctx.enter_context(tc.tile_pool(name="expp", bufs=6))
        sacp = ctx.enter_context(tc.tile_pool(name="sacp", bufs=2))
        mscp = ctx.enter_context(tc.tile_pool(name="mscp", bufs=3))
        smlp = ctx.enter_context(tc.tile_pool(name="smlp", bufs=2))
        outsp = ctx.enter_context(tc.tile_pool(name="outsp", bufs=4))
        psA = ctx.enter_context(tc.tile_pool(name="psA", bufs=3, space="PSUM"))
        psS = ctx.enter_context(tc.tile_pool(name="psS", bufs=2, space="PSUM"))
        psT = ctx.enter_context(tc.tile_pool(name="psT", bufs=2, space="PSUM"))
        psD = ctx.enter_context(tc.tile_pool(name="psD", bufs=1, space="PSUM"))

        # ---- constants (wq first: the first matmuls need it; wo last) ----
        wq_sb = const.tile([P, NKT, DSH], MMD)
        nc.sync.dma_start(out=wq_sb, in_=t["wqt"].rearrange("(k p) m -> p k m", p=P))
        wk_sb = const.tile([P, NKT, DSH], MMD)
        nc.gpsimd.dma_start(out=wk_sb, in_=t["wkt"].rearrange("(k p) m -> p k m", p=P))
        wv_sb = const.tile([P, NKT, DSH], MMD)
        nc.gpsimd.dma_start(out=wv_sb, in_=t["wvt"].rearrange("(k p) m -> p k m", p=P))
        tri_sb = const.tile([P, P], F32)
        nc.gpsimd.dma_start(out=tri_sb, in_=t["tri"])
        bq_sb = const.tile([P, HPC, 1], F32)
        nc.gpsimd.dma_start(out=bq_sb, in_=t["bqs"].rearrange("h p o -> p h o"))
        bk_sb = const.tile([P, HPC, 1], F32)
        nc.gpsimd.dma_start(out=bk_sb, in_=t["bks"].rearrange("h p o -> p h o"))
        ones2 = const.tile([P, 2], F32R)
        nc.gpsimd.dma_start(out=ones2, in_=t["onesc"])
        wo_sb = const.tile([P, HPC, D], MMD)
        nc.gpsimd.dma_start(out=wo_sb, in_=t["wot"].rearrange("(h p) n -> p h n", p=P))

        xt_view = t["xt"].rearrange("(k p) (c q) -> c p k q", p=P, q=QCW)

        for b in range(B):
            k_sb = kvp.tile([P, HPC, S], MMD, tag="k")
            v_sb = kvp.tile([P, NKT, DSH], MMD, tag="v")
            for qc in range(NQC):
                c = b * NQC + qc

                # ---- QKV projections for token chunk c ----
                xt0 = xtp.tile([P, NKT // 2, QCW], MMD, tag="xt")
                xt1 = xtp.tile([P, NKT // 2, QCW], MMD, tag="xt")
                nc.sync.dma_start(out=xt0, in_=xt_view[c][:, 0:8, :])
                nc.sync.dma_start(out=xt1, in_=xt_view[c][:, 8:16, :])
                xth = (xt0, xt1)
                q_sb = qch.tile([P, HPC, QCW], MMD, tag="q")
                for j in range(HPC):
                    qp = psA.tile([P, QCW], F32, tag="ps")
                    for k in range(NKT):
                        nc.tensor.matmul(
                            qp, wq_sb[:, k, j * DH:(j + 1) * DH],
                            xth[k // 8][:, k % 8, :],
                            start=(k == 0), stop=(k == NKT - 1))
                    nc.scalar.activation(q_sb[:, j, :], qp, AFT.Identity,
                                         bias=bq_sb[:, j, :], scale=SCALE)
                    kp = psA.tile([P, QCW], F32, tag="ps")
                    for k in range(NKT):
                        nc.tensor.matmul(
                            kp, wk_sb[:, k, j * DH:(j + 1) * DH],
                            xth[k // 8][:, k % 8, :],
                            start=(k == 0), stop=(k == NKT - 1))
                    nc.scalar.activation(k_sb[:, j, qc * QCW:(qc + 1) * QCW], kp,
                                         AFT.Identity, bias=bk_sb[:, j, :], scale=SCALE)
                for tt in range(KTPC):
                    vp = psA.tile([P, QCW], F32, tag="ps")
                    for k in range(NKT):
                        nc.tensor.matmul(
                            vp[:, 0:DSH],
                            xth[k // 8][:, k % 8, tt * P:(tt + 1) * P],
                            wv_sb[:, k, :],
                            start=(k == 0), stop=(k == NKT - 1))
                    nc.scalar.copy(v_sb[:, qc * KTPC + tt, :], vp[:, 0:DSH])

                # ---- causal attention for q-chunk qc, both heads ----
                a_sb = ach.tile([P, HPC, QCW], MMD, tag="a")
                rc_tiles = {}
                nkt_q = (qc + 1) * KTPC
                for h in range(HPC):
                    sacc = sacp.tile([P, QCW], F32, tag="sacc")
                    at = psT.tile([P, QCW], F32, tag="at")
                    for kt in range(nkt_q):
                        tdiag = kt - qc * KTPC
                        off = max(tdiag, 0) * P
                        w = QCW - off
                        sp = psS.tile([P, QCW], F32, tag="sc")
                        nc.tensor.matmul(
                            sp[:, 0:w], k_sb[:, h, kt * P:(kt + 1) * P],
                            q_sb[:, h, off:QCW], start=True, stop=True)
                        et = expp.tile([P, QCW], MMD, tag="exp")
                        if tdiag >= 0:
                            msc = mscp.tile([P, P], F32, tag="msc")
                            nc.vector.tensor_add(msc, sp[:, 0:P], tri_sb)
                            nc.scalar.activation(et[:, 0:P], msc, AFT.Exp)
                            if w > P:
                                nc.scalar.activation(et[:, P:w], sp[:, P:w], AFT.Exp)
                        else:
                            nc.scalar.activation(et[:, 0:w], sp[:, 0:w], AFT.Exp)
                        if kt == 0:
                            nc.vector.tensor_copy(sacc, et)
                        else:
                            nc.vector.tensor_add(sacc[:, off:QCW], sacc[:, off:QCW],
                                                 et[:, 0:w])
                        nc.tensor.matmul(
                            at[:, off:QCW], v_sb[:, kt, h * DH:(h + 1) * DH],
                            et[:, 0:w], start=(kt == 0), stop=(kt == nkt_q - 1))
                    nc.scalar.copy(a_sb[:, h, :], at)
                    sacc_r = smlp.tile([P, QCW], F32R, tag="saccr", bufs=2)
                    nc.scalar.copy(sacc_r, sacc)
                    dn = psD.tile([P, 2 * KTPC], F32, tag="dn")
                    for tt in range(KTPC):
                        nc.tensor.matmul(dn[:, 2 * tt:2 * tt + 2],
                                         sacc_r[:, tt * P:(tt + 1) * P], ones2,
                                         start=True, stop=True)
                    rc = smlp.tile([P, 2 * KTPC], F32, tag="rc", bufs=4)
                    nc.vector.reciprocal(rc, dn)
                    rc_tiles[h] = rc

                # ---- partial out-projection for this chunk's tokens ----
                for tt in range(KTPC):
                    r0 = rc_tiles[0][:, 2 * tt:2 * tt + 1]
                    r1 = rc_tiles[1][:, 2 * tt:2 * tt + 1]
                    for jc in range(NJC):
                        p0 = psA.tile([P, QCW], F32, tag="ps")
                        nc.tensor.matmul(p0, a_sb[:, 0, tt * P:(tt + 1) * P],
                                         wo_sb[:, 0, jc * QCW:(jc + 1) * QCW],
                                         start=True, stop=True)
                        p1 = psA.tile([P, QCW], F32, tag="ps")
                        nc.tensor.matmul(p1, a_sb[:, 1, tt * P:(tt + 1) * P],
                                         wo_sb[:, 1, jc * QCW:(jc + 1) * QCW],
                                         start=True, stop=True)
                        tmp = outsp.tile([P, QCW], F32, tag="ot")
                        nc.scalar.activation(tmp, p0, AFT.Identity, scale=r0)
                        ot = outsp.tile([P, QCW], F32, tag="ot")
                        nc.vector.scalar_tensor_tensor(
                            ot, p1, r1, tmp,
                            mybir.AluOpType.mult, mybir.AluOpType.add)
                        row0 = b * S + (qc * KTPC + tt) * P
                        nc.sync.dma_start(
                            out=t["outp"][row0:row0 + P, jc * QCW:(jc + 1) * QCW],
                            in_=ot)


def _build():
    nc = bacc.Bacc("TRN2", target_bir_lowering=False, debug=False,
                   num_devices=N_CORES)
    t = {
        "xt": nc.dram_tensor("xt", [D, TOK], BF16, kind="ExternalInput").ap(),
        "wqt": nc.dram_tensor("wqt", [D, DSH], BF16, kind="ExternalInput").ap(),
        "wkt": nc.dram_tensor("wkt", [D, DSH], BF16, kind="ExternalInput").ap(),
        "wvt": nc.dram_tensor("wvt", [D, DSH], BF16, kind="ExternalInput").ap(),
        "wot": nc.dram_tensor("wot", [DSH, D], BF16, kind="ExternalInput").ap(),
        "bqs": nc.dram_tensor("bqs", [HPC, P, 1], F32, kind="ExternalInput").ap(),
        "bks": nc.dram_tensor("bks", [HPC, P, 1], F32, kind="ExternalInput").ap(),
        "tri": nc.dram_tensor("tri", [P, P], F32, kind="ExternalInput").ap(),
        "onesc": nc.dram_tensor("onesc", [P, 2], F32R, kind="ExternalInput").ap(),
        "outp": nc.dram_tensor("outp", [TOK, D], F32, kind="ExternalOutput").ap(),
    }
    with tile.TileContext(nc) as tc:
        _emit(tc, t)
    nc.compile()
    return nc


def _program():
    global _nc
    if _nc is None:
        _nc = _build()
    return _nc


def kernel(X, Wq, bq, Wk, bk, Wv, bv, Wo, bo):
    X = np.asarray(X, np.float32)
    Wq = np.asarray(Wq, np.float32)
    Wk = np.asarray(Wk, np.float32)
    Wv = np.asarray(Wv, np.float32)
    Wo = np.asarray(Wo, np.float32)
    bq = np.asarray(bq, np.float32)
    bk = np.asarray(bk, np.float32)
    bv = np.asarray(bv, np.float32)
    bo = np.asarray(bo, np.float32)

    nc = _program()

    XT = np.ascontiguousarray(X.reshape(TOK, D).T).astype(ml_dtypes.bfloat16)
    tri = np.where(np.arange(P)[:, None] <= np.arange(P)[None, :],
                   np.float32(0.0), np.float32(MASK_NEG)).astype(np.float32)
    ones_col = np.ones((P, 2), np.float32)

    in_maps = []
    for c in range(N_CORES):
        J = slice(c * DSH, (c + 1) * DSH)
        in_maps.append({
            "xt": XT,
            "wqt": np.ascontiguousarray(Wq[J, :].T).astype(ml_dtypes.bfloat16),
            "wkt": np.ascontiguousarray(Wk[J, :].T).astype(ml_dtypes.bfloat16),
            "wvt": np.ascontiguousarray(Wv[J, :].T).astype(ml_dtypes.bfloat16),
            "wot": np.ascontiguousarray(Wo[:, J].T).astype(ml_dtypes.bfloat16),
            "bqs": (bq[J] * SCALE).reshape(HPC, P, 1).astype(np.float32),
            "bks": (bk[J] * SCALE).reshape(HPC, P, 1).astype(np.float32),
            "tri": tri,
            "onesc": ones_col,
        })

    res = run_bass_kernel_spmd(nc, in_maps, list(range(N_CORES)), trace=TRACE)
    LAST["res"] = res

    out = res.results[0]["outp"].copy()
    for c in range(1, N_CORES):
        out += res.results[c]["outp"]
    out += (bo + Wo @ bv)[None, :].astype(np.float32)
    return out.reshape(B, S, D).astype(np.float32)


# revision 7
# speedup vs baseline: 1.1499x; 1.0414x over previous
"""Multi-headed causal self-attention on 8 Trainium2 NeuronCores.

Sharding: tensor-parallel over heads (2 of 16 heads per core).
Each core computes Q/K/V projections for its 256-wide feature slice,
causal attention for its 2 heads, and the partial output projection
through its slice of Wo.  The host sums the 8 partial outputs and adds
the analytically-folded constant row  bo + Wo @ bv  (softmax rows sum
to one, so V's bias contributes a constant vector through Wo).

On-chip layout (all matmuls in float32r = full PE rate):
  - X is passed host-pre-transposed as XT [D, B*S] so the contraction
    dim of every projection lands on SBUF partitions.
  - Q, K are produced feature-major [d_head, tok]; V token-major
    [tok, d_head].
  - scores are computed transposed (scoresT [k, q]) so that exp@V needs
    no transposes and softmax denominators come from a ones-matmul.
  - softmax skips max-subtraction (scores are bounded ~|5| for these
    input scales); causal masking adds -60 to invalid logits of
    diagonal 128x128 blocks before exp, off-diagonal blocks above the
    diagonal are simply never computed.
  - the per-token denominators are applied after the out-projection
    (per head), where they sit on the partition axis.
"""

import ml_dtypes
import numpy as np

import concourse.bass as bass  # noqa: F401  (registers engine types)
import concourse.tile as tile
from concourse import bacc, mybir
from concourse.bass_utils import run_bass_kernel_spmd

N_CORES = 8
B, S, D = 2, 2048, 2048
H, DH = 16, 128
HPC = H // N_CORES          # heads per core
DSH = HPC * DH              # per-core feature slice width (256)
TOK = B * S
P = 128                     # SBUF partitions
QCW = 512                   # query-chunk width (matmul moving dim)
NQC = S // QCW              # q-chunks per batch
NKT = S // P                # k-tiles per batch
KTPC = QCW // P             # k-tiles per q-chunk
NJC = D // QCW              # output column chunks
SCALE = float(1.0 / np.sqrt(np.sqrt(DH)))
MASK_NEG = -60.0

F32 = mybir.dt.float32
F32R = mybir.dt.float32r
BF16 = mybir.dt.bfloat16
MMD = BF16                  # matmul operand dtype (bf16 -> FWL weight loads)
AFT = mybir.ActivationFunctionType

TRACE = False
LAST = {}

_nc = None


def _emit(tc, t):
    from contextlib import ExitStack

    nc = tc.nc
    with ExitStack() as ctx:
        const = ctx.enter_context(tc.tile_pool(name="const", bufs=1))
        xtp = ctx.enter_context(tc.tile_pool(name="xtp", bufs=4))
        kvp = ctx.enter_context(tc.tile_pool(name="kvp", bufs=2))
        qch = ctx.enter_context(tc.tile_pool(name="qch", bufs=3))
        ach = ctx.enter_context(tc.tile_pool(name="ach", bufs=3))
        expp = ctx.enter_context(tc.tile_pool(name="expp", bufs=6))
        sacp = ctx.enter_context(tc.tile_pool(name="sacp", bufs=2))
        mscp = ctx.enter_context(tc.tile_pool(name="mscp", bufs=3))
        smlp = ctx.enter_context(tc.tile_pool(name="smlp", bufs=2))
        outsp = ctx.enter_context(tc.tile_pool(name="outsp", bufs=4))
        psA = ctx.enter_context(tc.tile_pool(name="psA", bufs=3, space="PSUM"))
        psS = ctx.enter_context(tc.tile_pool(name="psS", bufs=2, space="PSUM"))
        psT = ctx.enter_context(tc.tile_pool(name="psT", bufs=2, space="PSUM"))
        psD = ctx.enter_context(tc.tile_pool(name="psD", bufs=1, space="PSUM"))

        # ---- constants (wq first: the first matmuls need it; wo last) ----
        wq_sb = const.tile([P, NKT, DSH], MMD)
        nc.sync.dma_start(out=wq_sb, in_=t["wqt"].rearrange("(k p) m -> p k m", p=P))
        wk_sb = const.tile([P, NKT, DSH], MMD)
        nc.gpsimd.dma_start(out=wk_sb, in_=t["wkt"].rearrange("(k p) m -> p k m", p=P))
        wv_sb = const.tile([P, NKT, DSH], MMD)
        nc.gpsimd.dma_start(out=wv_sb, in_=t["wvt"].rearrange("(k p) m -> p k m", p=P))
        tri_sb = const.tile([P, P], F32)
        nc.gpsimd.dma_start(out=tri_sb, in_=t["tri"])
        bq_sb = const.tile([P, HPC, 1], F32)
        nc.gpsimd.dma_start(out=bq_sb, in_=t["bqs"].rearrange("h p o -> p h o"))
        bk_sb = const.tile([P, HPC, 1], F32)
        nc.gpsimd.dma_start(out=bk_sb, in_=t["bks"].rearrange("h p o -> p h o"))
        ones_mat = const.tile([P, P], MMD)
        nc.gpsimd.dma_start(out=ones_mat, in_=t["onesc"])
        wo_sb = const.tile([P, HPC, D], MMD)
        nc.gpsimd.dma_start(out=wo_sb, in_=t["wot"].rearrange("(h p) n -> p h n", p=P))

        xt_view = t["xt"].rearrange("(k p) (c q) -> c p k q", p=P, q=QCW)

        for b in range(B):
            k_sb = kvp.tile([P, HPC, S], MMD, tag="k")
            v_sb = kvp.tile([P, NKT, DSH], MMD, tag="v")
            for qc in range(NQC):
                c = b * NQC + qc

                # ---- QKV projections for token chunk c ----
                xt0 = xtp.tile([P, NKT // 2, QCW], MMD, tag="xt")
                xt1 = xtp.tile([P, NKT // 2, QCW], MMD, tag="xt")
                nc.sync.dma_start(out=xt0, in_=xt_view[c][:, 0:8, :])
                nc.sync.dma_start(out=xt1, in_=xt_view[c][:, 8:16, :])
                xth = (xt0, xt1)
                q_sb = qch.tile([P, HPC, QCW], MMD, tag="q")
                for j in range(HPC):
                    qp = psA.tile([P, QCW], F32, tag="ps")
                    for k in range(NKT):
                        nc.tensor.matmul(
                            qp, wq_sb[:, k, j * DH:(j + 1) * DH],
                            xth[k // 8][:, k % 8, :],
                            start=(k == 0), stop=(k == NKT - 1))
                    nc.scalar.activation(q_sb[:, j, :], qp, AFT.Identity,
                                         bias=bq_sb[:, j, :], scale=SCALE)
                    kp = psA.tile([P, QCW], F32, tag="ps")
                    for k in range(NKT):
                        nc.tensor.matmul(
                            kp, wk_sb[:, k, j * DH:(j + 1) * DH],
                            xth[k // 8][:, k % 8, :],
                            start=(k == 0), stop=(k == NKT - 1))
                    nc.scalar.activation(k_sb[:, j, qc * QCW:(qc + 1) * QCW], kp,
                                         AFT.Identity, bias=bk_sb[:, j, :], scale=SCALE)
                for tt in range(KTPC):
                    vp = psA.tile([P, QCW], F32, tag="ps")
                    for k in range(NKT):
                        nc.tensor.matmul(
                            vp[:, 0:DSH],
                            xth[k // 8][:, k % 8, tt * P:(tt + 1) * P],
                            wv_sb[:, k, :],
                            start=(k == 0), stop=(k == NKT - 1))
                    nc.vector.tensor_copy(v_sb[:, qc * KTPC + tt, :], vp[:, 0:DSH])

                # ---- causal attention for q-chunk qc, both heads ----
                a_sb = ach.tile([P, HPC, QCW], MMD, tag="a")
                nkt_q = (qc + 1) * KTPC
                for h in range(HPC):
                    sacc = sacp.tile([P, QCW], F32, tag="sacc")
                    at = psT.tile([P, QCW], F32, tag="at")
                    for kt in range(nkt_q):
                        tdiag = kt - qc * KTPC
                        off = max(tdiag, 0) * P
                        w = QCW - off
                        sp = psS.tile([P, QCW], F32, tag="sc")
                        nc.tensor.matmul(
                            sp[:, 0:w], k_sb[:, h, kt * P:(kt + 1) * P],
                            q_sb[:, h, off:QCW], start=True, stop=True)
                        et = expp.tile([P, QCW], MMD, tag="exp")
                        if tdiag >= 0:
                            msc = mscp.tile([P, P], F32, tag="msc")
                            nc.vector.tensor_add(msc, sp[:, 0:P], tri_sb)
                            nc.scalar.activation(et[:, 0:P], msc, AFT.Exp)
                            if w > P:
                                nc.scalar.activation(et[:, P:w], sp[:, P:w], AFT.Exp)
                        else:
                            nc.scalar.activation(et[:, 0:w], sp[:, 0:w], AFT.Exp)
                        if kt == 0:
                            nc.vector.tensor_copy(sacc, et)
                        else:
                            nc.vector.tensor_add(sacc[:, off:QCW], sacc[:, off:QCW],
                                                 et[:, 0:w])
                        nc.tensor.matmul(
                            at[:, off:QCW], v_sb[:, kt, h * DH:(h + 1) * DH],
                            et[:, 0:w], start=(kt == 0), stop=(kt == nkt_q - 1))
                    sacc_r = smlp.tile([P, QCW], MMD, tag="saccr", bufs=2)
                    nc.scalar.copy(sacc_r, sacc)
                    dnb = psD.tile([P, QCW], F32, tag="dn")
                    nc.tensor.matmul(dnb, ones_mat, sacc_r, start=True, stop=True)
                    rcf = smlp.tile([P, QCW], F32, tag="rcf", bufs=2)
                    nc.vector.reciprocal(rcf, dnb)
                    nc.vector.tensor_mul(a_sb[:, h, :], at, rcf)

                # ---- partial out-projection for this chunk's tokens ----
                for tt in range(KTPC):
                    for jc in range(NJC):
                        po = psA.tile([P, QCW], F32, tag="ps")
                        for h in range(HPC):
                            nc.tensor.matmul(po, a_sb[:, h, tt * P:(tt + 1) * P],
                                             wo_sb[:, h, jc * QCW:(jc + 1) * QCW],
                                             start=(h == 0), stop=(h == HPC - 1))
                        ot = outsp.tile([P, QCW], F32, tag="ot")
                        if (tt + jc) % 2 == 0:
                            nc.vector.tensor_copy(ot, po)
                        else:
                            nc.scalar.copy(ot, po)
                        row0 = b * S + (qc * KTPC + tt) * P
                        nc.sync.dma_start(
                            out=t["outp"][row0:row0 + P, jc * QCW:(jc + 1) * QCW],
                            in_=ot)


def _build():
    nc = bacc.Bacc("TRN2", target_bir_lowering=False, debug=False,
                   num_devices=N_CORES)
    t = {
        "xt": nc.dram_tensor("xt", [D, TOK], BF16, kind="ExternalInput").ap(),
        "wqt": nc.dram_tensor("wqt", [D, DSH], BF16, kind="ExternalInput").ap(),
        "wkt": nc.dram_tensor("wkt", [D, DSH], BF16, kind="ExternalInput").ap(),
        "wvt": nc.dram_tensor("wvt", [D, DSH], BF16, kind="ExternalInput").ap(),
        "wot": nc.dram_tensor("wot", [DSH, D], BF16, kind="ExternalInput").ap(),
        "bqs": nc.dram_tensor("bqs", [HPC, P, 1], F32, kind="ExternalInput").ap(),
        "bks": nc.dram_tensor("bks", [HPC, P, 1], F32, kind="ExternalInput").ap(),
        "tri": nc.dram_tensor("tri", [P, P], F32, kind="ExternalInput").ap(),
        "onesc": nc.dram_tensor("onesc", [P, P], BF16, kind="ExternalInput").ap(),
        "outp": nc.dram_tensor("outp", [TOK, D], F32, kind="ExternalOutput").ap(),
    }
    with tile.TileContext(nc) as tc:
        _emit(tc, t)
    nc.compile()
    return nc


def _program():
    global _nc
    if _nc is None:
        _nc = _build()
    return _nc


def kernel(X, Wq, bq, Wk, bk, Wv, bv, Wo, bo):
    X = np.asarray(X, np.float32)
    Wq = np.asarray(Wq, np.float32)
    Wk = np.asarray(Wk, np.float32)
    Wv = np.asarray(Wv, np.float32)
    Wo = np.asarray(Wo, np.float32)
    bq = np.asarray(bq, np.float32)
    bk = np.asarray(bk, np.float32)
    bv = np.asarray(bv, np.float32)
    bo = np.asarray(bo, np.float32)

    nc = _program()

    XT = np.ascontiguousarray(X.reshape(TOK, D).T).astype(ml_dtypes.bfloat16)
    tri = np.where(np.arange(P)[:, None] <= np.arange(P)[None, :],
                   np.float32(0.0), np.float32(MASK_NEG)).astype(np.float32)
    ones_col = np.ones((P, P), ml_dtypes.bfloat16)

    in_maps = []
    for c in range(N_CORES):
        J = slice(c * DSH, (c + 1) * DSH)
        in_maps.append({
            "xt": XT,
            "wqt": np.ascontiguousarray(Wq[J, :].T).astype(ml_dtypes.bfloat16),
            "wkt": np.ascontiguousarray(Wk[J, :].T).astype(ml_dtypes.bfloat16),
            "wvt": np.ascontiguousarray(Wv[J, :].T).astype(ml_dtypes.bfloat16),
            "wot": np.ascontiguousarray(Wo[:, J].T).astype(ml_dtypes.bfloat16),
            "bqs": (bq[J] * SCALE).reshape(HPC, P, 1).astype(np.float32),
            "bks": (bk[J] * SCALE).reshape(HPC, P, 1).astype(np.float32),
            "tri": tri,
            "onesc": ones_col,
        })

    res = run_bass_kernel_spmd(nc, in_maps, list(range(N_CORES)), trace=TRACE)
    LAST["res"] = res

    out = res.results[0]["outp"].copy()
    for c in range(1, N_CORES):
        out += res.results[c]["outp"]
    out += (bo + Wo @ bv)[None, :].astype(np.float32)
    return out.reshape(B, S, D).astype(np.float32)


# revision 16
# speedup vs baseline: 1.3955x; 1.2135x over previous
"""Multi-headed causal self-attention on 8 Trainium2 NeuronCores.

Sharding: tensor-parallel over heads (2 of 16 heads per core).
Each core computes Q/K/V projections for its 256-wide feature slice,
causal attention for its 2 heads, and the partial output projection
through its slice of Wo.  The host sums the 8 partial outputs and adds
the analytically-folded constant row  bo + Wo @ bv  (softmax rows sum
to one, so V's bias contributes a constant vector through Wo).

On-chip layout (all matmuls in float32r = full PE rate):
  - X is passed host-pre-transposed as XT [D, B*S] so the contraction
    dim of every projection lands on SBUF partitions.
  - Q, K are produced feature-major [d_head, tok]; V token-major
    [tok, d_head].
  - scores are computed transposed (scoresT [k, q]) so that exp@V needs
    no transposes and softmax denominators come from a ones-matmul.
  - softmax skips max-subtraction (scores are bounded ~|5| for these
    input scales); causal masking adds -60 to invalid logits of
    diagonal 128x128 blocks before exp, off-diagonal blocks above the
    diagonal are simply never computed.
  - the per-token denominators are applied after the out-projection
    (per head), where they sit on the partition axis.
"""

import ml_dtypes
import numpy as np

import concourse.bass as bass  # noqa: F401  (registers engine types)
import concourse.tile as tile
from concourse import bacc, mybir
from concourse.bass_utils import run_bass_kernel_spmd


N_CORES = 8
B, S, D = 2, 2048, 2048
H, DH = 16, 128
HPC = H // N_CORES          # heads per core
DSH = HPC * DH              # per-core feature slice width (256)
TOK = B * S
P = 128                     # SBUF partitions
QCW = 512                   # query-chunk width (matmul moving dim)
NQC = S // QCW              # q-chunks per batch
NKT = S // P                # k-tiles per batch
KTPC = QCW // P             # k-tiles per q-chunk
NJC = D // QCW              # output column chunks
SCALE = float(1.0 / np.sqrt(np.sqrt(DH)))
MASK_NEG = -60.0

F32 = mybir.dt.float32
F32R = mybir.dt.float32r
BF16 = mybir.dt.bfloat16
MMD = BF16                  # matmul operand dtype (bf16 -> FWL weight loads)
AFT = mybir.ActivationFunctionType

TRACE = False
LAST = {}

_nc = None


def _emit(tc, t):
    from contextlib import ExitStack

    nc = tc.nc
    with ExitStack() as ctx:
        const = ctx.enter_context(tc.tile_pool(name="const", bufs=1))
        xtp = ctx.enter_context(tc.tile_pool(name="xtp", bufs=6))
        kvp = ctx.enter_context(tc.tile_pool(name="kvp", bufs=2))
        qch = ctx.enter_context(tc.tile_pool(name="qch", bufs=3))
        ach = ctx.enter_context(tc.tile_pool(name="ach", bufs=3))
        expp = ctx.enter_context(tc.tile_pool(name="expp", bufs=6))
        sacp = ctx.enter_context(tc.tile_pool(name="sacp", bufs=2))
        mscp = ctx.enter_context(tc.tile_pool(name="mscp", bufs=3))
        smlp = ctx.enter_context(tc.tile_pool(name="smlp", bufs=2))
        outsp = ctx.enter_context(tc.tile_pool(name="outsp", bufs=4))
        psA = ctx.enter_context(tc.tile_pool(name="psA", bufs=2, space="PSUM"))
        psS = ctx.enter_context(tc.tile_pool(name="psS", bufs=3, space="PSUM"))
        psT = ctx.enter_context(tc.tile_pool(name="psT", bufs=2, space="PSUM"))
        psD = ctx.enter_context(tc.tile_pool(name="psD", bufs=1, space="PSUM"))

        # ---- constants (wq first: the first matmuls need it; wo last) ----
        wq_sb = const.tile([P, NKT, DSH], MMD)
        _wqv = t["wqt"].rearrange("(k p) m -> p k m", p=P)
        nc.sync.dma_start(out=wq_sb[:, 0:4, :], in_=_wqv[:, 0:4, :])
        wk_sb = const.tile([P, NKT, DSH], MMD)
        nc.gpsimd.dma_start(out=wk_sb, in_=t["wkt"].rearrange("(k p) m -> p k m", p=P))
        wv_sb = const.tile([P, NKT, DSH], MMD)
        nc.gpsimd.dma_start(out=wv_sb, in_=t["wvt"].rearrange("(k p) m -> p k m", p=P))
        tri_sb = const.tile([P, P], F32)
        nc.gpsimd.dma_start(out=tri_sb, in_=t["tri"])
        bq_sb = const.tile([P, HPC, 1], F32)
        nc.gpsimd.dma_start(out=bq_sb, in_=t["bqs"].rearrange("h p o -> p h o"))
        bk_sb = const.tile([P, HPC, 1], F32)
        nc.gpsimd.dma_start(out=bk_sb, in_=t["bks"].rearrange("h p o -> p h o"))
        ones_mat = const.tile([P, P], MMD)
        nc.gpsimd.dma_start(out=ones_mat, in_=t["onesc"])
        wo_sb = const.tile([P, HPC, D], MMD)
        nc.gpsimd.dma_start(out=wo_sb, in_=t["wot"].rearrange("(h p) n -> p h n", p=P))

        xt_view = t["xt"].rearrange("(k p) (c q) -> c p k q", p=P, q=QCW)

        def outproj_unit(bp, qcp, a_prev, tt, jc):
            # one [128-token x 512-col] slab of the previous chunk's
            # out-projection; woven between attention blocks to keep PE fed
            po = psA.tile([P, QCW], F32, tag="ps")
            for h in range(HPC):
                nc.tensor.matmul(po, a_prev[:, h, tt * P:(tt + 1) * P],
                                 wo_sb[:, h, jc * QCW:(jc + 1) * QCW],
                                 start=(h == 0), stop=(h == HPC - 1))
            ot = outsp.tile([P, QCW], F32, tag="ot")
            nc.vector.tensor_copy(ot, po)
            row0 = bp * S + (qcp * KTPC + tt) * P
            nc.sync.dma_start(
                out=t["outp"][row0:row0 + P, jc * QCW:(jc + 1) * QCW], in_=ot)

        prev = None  # (b, qc, a_sb) of the chunk whose out-proj is pending
        for b in range(B):
            k_sb = kvp.tile([P, HPC, S], MMD, tag="k")
            v_sb = kvp.tile([P, NKT, DSH], MMD, tag="v")
            for qc in range(NQC):
                c = b * NQC + qc

                # ---- QKV projections for token chunk c ----
                xt0 = xtp.tile([P, NKT // 2, QCW], MMD, tag="xt")
                xt1 = xtp.tile([P, NKT // 2, QCW], MMD, tag="xt")
                if c == 0:
                    # interleave with the remaining wq pieces so the first
                    # Q-projection matmuls start as early as possible
                    nc.sync.dma_start(out=xt0[:, 0:4, :], in_=xt_view[0][:, 0:4, :])
                    nc.sync.dma_start(out=wq_sb[:, 4:16, :], in_=_wqv[:, 4:16, :])
                    nc.sync.dma_start(out=xt0[:, 4:8, :], in_=xt_view[0][:, 4:8, :])
                    nc.sync.dma_start(out=xt1, in_=xt_view[0][:, 8:16, :])
                else:
                    nc.sync.dma_start(out=xt0, in_=xt_view[c][:, 0:8, :])
                    nc.sync.dma_start(out=xt1, in_=xt_view[c][:, 8:16, :])
                xth = (xt0, xt1)
                q_sb = qch.tile([P, HPC, QCW], MMD, tag="q")
                for j in range(HPC):
                    qp = psA.tile([P, QCW], F32, tag="ps")
                    for k in range(NKT):
                        nc.tensor.matmul(
                            qp, wq_sb[:, k, j * DH:(j + 1) * DH],
                            xth[k // 8][:, k % 8, :],
                            start=(k == 0), stop=(k == NKT - 1))
                    nc.scalar.activation(q_sb[:, j, :], qp, AFT.Identity,
                                         bias=bq_sb[:, j, :], scale=SCALE)
                    kp = psA.tile([P, QCW], F32, tag="ps")
                    for k in range(NKT):
                        nc.tensor.matmul(
                            kp, wk_sb[:, k, j * DH:(j + 1) * DH],
                            xth[k // 8][:, k % 8, :],
                            start=(k == 0), stop=(k == NKT - 1))
                    nc.scalar.activation(k_sb[:, j, qc * QCW:(qc + 1) * QCW], kp,
                                         AFT.Identity, bias=bk_sb[:, j, :], scale=SCALE)
                for tt in range(KTPC):
                    vp = psA.tile([P, QCW], F32, tag="ps")
                    for k in range(NKT):
                        nc.tensor.matmul(
                            vp[:, 0:DSH],
                            xth[k // 8][:, k % 8, tt * P:(tt + 1) * P],
                            wv_sb[:, k, :],
                            start=(k == 0), stop=(k == NKT - 1))
                    nc.vector.tensor_copy(v_sb[:, qc * KTPC + tt, :], vp[:, 0:DSH])

                # ---- causal attention for q-chunk qc, both heads, with the
                # previous chunk's out-projection interleaved ----
                a_sb = ach.tile([P, HPC, QCW], MMD, tag="a")
                nkt_q = (qc + 1) * KTPC
                units = ([(tt, jc) for tt in range(KTPC) for jc in range(NJC)]
                         if prev is not None else [])
                ui = 0
                for h in range(HPC):
                    sacc = sacp.tile([P, QCW], F32, tag="sacc")
                    at = psT.tile([P, QCW], F32, tag="at")
                    for kt in range(nkt_q):
                        tdiag = kt - qc * KTPC
                        off = max(tdiag, 0) * P
                        w = QCW - off
                        sp = psS.tile([P, QCW], F32, tag="sc")
                        nc.tensor.matmul(
                            sp[:, 0:w], k_sb[:, h, kt * P:(kt + 1) * P],
                            q_sb[:, h, off:QCW], start=True, stop=True)
                        if tdiag >= 0:
                            nc.vector.tensor_add(sp[:, 0:P], sp[:, 0:P], tri_sb)
                        et = expp.tile([P, QCW], MMD, tag="exp")
                        nc.scalar.activation(et[:, 0:w], sp[:, 0:w], AFT.Exp)
                        if kt == 0:
                            nc.vector.tensor_copy(sacc, et)
                        else:
                            nc.vector.tensor_add(sacc[:, off:QCW], sacc[:, off:QCW],
                                                 et[:, 0:w])
                        nc.tensor.matmul(
                            at[:, off:QCW], v_sb[:, kt, h * DH:(h + 1) * DH],
                            et[:, 0:w], start=(kt == 0), stop=(kt == nkt_q - 1))
                        if ui < len(units):
                            outproj_unit(prev[0], prev[1], prev[2], *units[ui])
                            ui += 1
                    sacc_r = smlp.tile([P, QCW], MMD, tag="saccr", bufs=2)
                    nc.scalar.copy(sacc_r, sacc)
                    dnb = psD.tile([P, QCW], F32, tag="dn")
                    nc.tensor.matmul(dnb, ones_mat, sacc_r, start=True, stop=True)
                    rcf = smlp.tile([P, QCW], F32, tag="rcf", bufs=2)
                    nc.vector.reciprocal(rcf, dnb)
                    nc.vector.tensor_mul(a_sb[:, h, :], at, rcf)
                while ui < len(units):
                    outproj_unit(prev[0], prev[1], prev[2], *units[ui])
                    ui += 1
                prev = (b, qc, a_sb)

        # flush the final chunk's out-projection
        for tt in range(KTPC):
            for jc in range(NJC):
                outproj_unit(prev[0], prev[1], prev[2], tt, jc)


def _build():
    nc = bacc.Bacc("TRN2", target_bir_lowering=False, debug=False,
                   num_devices=N_CORES)
    t = {
        "xt": nc.dram_tensor("xt", [D, TOK], BF16, kind="ExternalInput").ap(),
        "wqt": nc.dram_tensor("wqt", [D, DSH], BF16, kind="ExternalInput").ap(),
        "wkt": nc.dram_tensor("wkt", [D, DSH], BF16, kind="ExternalInput").ap(),
        "wvt": nc.dram_tensor("wvt", [D, DSH], BF16, kind="ExternalInput").ap(),
        "wot": nc.dram_tensor("wot", [DSH, D], BF16, kind="ExternalInput").ap(),
        "bqs": nc.dram_tensor("bqs", [HPC, P, 1], F32, kind="ExternalInput").ap(),
        "bks": nc.dram_tensor("bks", [HPC, P, 1], F32, kind="ExternalInput").ap(),
        "tri": nc.dram_tensor("tri", [P, P], F32, kind="ExternalInput").ap(),
        "onesc": nc.dram_tensor("onesc", [P, P], BF16, kind="ExternalInput").ap(),
        "outp": nc.dram_tensor("outp", [TOK, D], F32, kind="ExternalOutput").ap(),
    }
    with tile.TileContext(nc) as tc:
        _emit(tc, t)
    nc.compile()
    return nc


def _program():
    global _nc
    if _nc is None:
        _nc = _build()
    return _nc


def kernel(X, Wq, bq, Wk, bk, Wv, bv, Wo, bo):
    X = np.asarray(X, np.float32)
    Wq = np.asarray(Wq, np.float32)
    Wk = np.asarray(Wk, np.float32)
    Wv = np.asarray(Wv, np.float32)
    Wo = np.asarray(Wo, np.float32)
    bq = np.asarray(bq, np.float32)
    bk = np.asarray(bk, np.float32)
    bv = np.asarray(bv, np.float32)
    bo = np.asarray(bo, np.float32)

    nc = _program()

    XT = np.ascontiguousarray(X.reshape(TOK, D).T).astype(ml_dtypes.bfloat16)
    tri = np.where(np.arange(P)[:, None] <= np.arange(P)[None, :],
                   np.float32(0.0), np.float32(MASK_NEG)).astype(np.float32)
    ones_col = np.ones((P, P), ml_dtypes.bfloat16)

    in_maps = []
    for c in range(N_CORES):
        J = slice(c * DSH, (c + 1) * DSH)
        in_maps.append({
            "xt": XT,
            "wqt": np.ascontiguousarray(Wq[J, :].T).astype(ml_dtypes.bfloat16),
            "wkt": np.ascontiguousarray(Wk[J, :].T).astype(ml_dtypes.bfloat16),
            "wvt": np.ascontiguousarray(Wv[J, :].T).astype(ml_dtypes.bfloat16),
            "wot": np.ascontiguousarray(Wo[:, J].T).astype(ml_dtypes.bfloat16),
            "bqs": (bq[J] * SCALE).reshape(HPC, P, 1).astype(np.float32),
            "bks": (bk[J] * SCALE).reshape(HPC, P, 1).astype(np.float32),
            "tri": tri,
            "onesc": ones_col,
        })

    res = run_bass_kernel_spmd(nc, in_maps, list(range(N_CORES)), trace=TRACE)
    LAST["res"] = res

    out = res.results[0]["outp"].copy()
    for c in range(1, N_CORES):
        out += res.results[c]["outp"]
    out += (bo + Wo @ bv)[None, :].astype(np.float32)
    return out.reshape(B, S, D).astype(np.float32)


# revision 18
# speedup vs baseline: 1.5439x; 1.1063x over previous
"""Multi-headed causal self-attention on 8 Trainium2 NeuronCores.

Sharding: tensor-parallel over heads (2 of 16 heads per core).
Each core computes Q/K/V projections for its 256-wide feature slice,
causal attention for its 2 heads, and the partial output projection
through its slice of Wo.  The host sums the 8 partial outputs and adds
the analytically-folded constant row  bo + Wo @ bv  (softmax rows sum
to one, so V's bias contributes a constant vector through Wo).

On-chip layout (all matmuls in float32r = full PE rate):
  - X is passed host-pre-transposed as XT [D, B*S] so the contraction
    dim of every projection lands on SBUF partitions.
  - Q, K are produced feature-major [d_head, tok]; V token-major
    [tok, d_head].
  - scores are computed transposed (scoresT [k, q]) so that exp@V needs
    no transposes and softmax denominators come from a ones-matmul.
  - softmax skips max-subtraction (scores are bounded ~|5| for these
    input scales); causal masking adds -60 to invalid logits of
    diagonal 128x128 blocks before exp, off-diagonal blocks above the
    diagonal are simply never computed.
  - the per-token denominators are applied after the out-projection
    (per head), where they sit on the partition axis.
"""

import ml_dtypes
import numpy as np

import concourse.bass as bass  # noqa: F401  (registers engine types)
import concourse.tile as tile
from concourse import bacc, mybir
from concourse.bass_utils import run_bass_kernel_spmd


N_CORES = 8
B, S, D = 2, 2048, 2048
H, DH = 16, 128
HPC = H // N_CORES          # heads per core
DSH = HPC * DH              # per-core feature slice width (256)
TOK = B * S
P = 128                     # SBUF partitions
QCW = 512                   # query-chunk width (matmul moving dim)
NQC = S // QCW              # q-chunks per batch
NKT = S // P                # k-tiles per batch
KTPC = QCW // P             # k-tiles per q-chunk
NJC = D // QCW              # output column chunks
SCALE = float(1.0 / np.sqrt(np.sqrt(DH)))
MASK_NEG = -60.0

F32 = mybir.dt.float32
F32R = mybir.dt.float32r
BF16 = mybir.dt.bfloat16
MMD = BF16                  # matmul operand dtype (bf16 -> FWL weight loads)
AFT = mybir.ActivationFunctionType

TRACE = False
LAST = {}

_nc = None


def _emit(tc, t):
    from contextlib import ExitStack

    nc = tc.nc
    with ExitStack() as ctx:
        const = ctx.enter_context(tc.tile_pool(name="const", bufs=1))
        xtp = ctx.enter_context(tc.tile_pool(name="xtp", bufs=6))
        kvp = ctx.enter_context(tc.tile_pool(name="kvp", bufs=2))
        qch = ctx.enter_context(tc.tile_pool(name="qch", bufs=3))
        ach = ctx.enter_context(tc.tile_pool(name="ach", bufs=3))
        expp = ctx.enter_context(tc.tile_pool(name="expp", bufs=8))
        sacp = ctx.enter_context(tc.tile_pool(name="sacp", bufs=2))
        mscp = ctx.enter_context(tc.tile_pool(name="mscp", bufs=3))
        smlp = ctx.enter_context(tc.tile_pool(name="smlp", bufs=2))
        outsp = ctx.enter_context(tc.tile_pool(name="outsp", bufs=8))
        psA = ctx.enter_context(tc.tile_pool(name="psA", bufs=2, space="PSUM"))
        psS = ctx.enter_context(tc.tile_pool(name="psS", bufs=3, space="PSUM"))
        psT = ctx.enter_context(tc.tile_pool(name="psT", bufs=2, space="PSUM"))
        psD = ctx.enter_context(tc.tile_pool(name="psD", bufs=1, space="PSUM"))

        # ---- constants (wq first: the first matmuls need it; wo last) ----
        wq_sb = const.tile([P, NKT, DSH], MMD)
        _wqv = t["wqt"].rearrange("(k p) m -> p k m", p=P)
        nc.sync.dma_start(out=wq_sb[:, 0:4, :], in_=_wqv[:, 0:4, :])
        wk_sb = const.tile([P, NKT, DSH], MMD)
        nc.gpsimd.dma_start(out=wk_sb, in_=t["wkt"].rearrange("(k p) m -> p k m", p=P))
        wv_sb = const.tile([P, NKT, DSH], MMD)
        nc.gpsimd.dma_start(out=wv_sb, in_=t["wvt"].rearrange("(k p) m -> p k m", p=P))
        tri_sb = const.tile([P, P], F32)
        nc.gpsimd.dma_start(out=tri_sb, in_=t["tri"])
        bq_sb = const.tile([P, HPC, 1], F32)
        nc.gpsimd.dma_start(out=bq_sb, in_=t["bqs"].rearrange("h p o -> p h o"))
        bk_sb = const.tile([P, HPC, 1], F32)
        nc.gpsimd.dma_start(out=bk_sb, in_=t["bks"].rearrange("h p o -> p h o"))
        ones_mat = const.tile([P, P], MMD)
        nc.gpsimd.dma_start(out=ones_mat, in_=t["onesc"])
        wo_sb = const.tile([P, HPC, D], MMD)
        nc.gpsimd.dma_start(out=wo_sb, in_=t["wot"].rearrange("(h p) n -> p h n", p=P))

        xt_view = t["xt"].rearrange("(k p) (c q) -> c p k q", p=P, q=QCW)

        def outproj_unit(bp, qcp, a_prev, tt, jc):
            # one [128-token x 512-col] slab of the previous chunk's
            # out-projection; woven between attention blocks to keep PE fed
            po = psA.tile([P, QCW], F32, tag="ps")
            for h in range(HPC):
                nc.tensor.matmul(po, a_prev[:, h, tt * P:(tt + 1) * P],
                                 wo_sb[:, h, jc * QCW:(jc + 1) * QCW],
                                 start=(h == 0), stop=(h == HPC - 1))
            ot = outsp.tile([P, QCW], F32, tag="ot")
            if (tt + jc) % 2 == 0:
                nc.vector.tensor_copy(ot, po)
            else:
                nc.scalar.copy(ot, po)
            row0 = bp * S + (qcp * KTPC + tt) * P
            nc.sync.dma_start(
                out=t["outp"][row0:row0 + P, jc * QCW:(jc + 1) * QCW], in_=ot)

        prev = None  # (b, qc, a_sb) of the chunk whose out-proj is pending
        for b in range(B):
            k_sb = kvp.tile([P, HPC, S], MMD, tag="k")
            v_sb = kvp.tile([P, NKT, DSH], MMD, tag="v")
            for qc in range(NQC):
                c = b * NQC + qc

                # ---- QKV projections for token chunk c ----
                xt0 = xtp.tile([P, NKT // 2, QCW], MMD, tag="xt")
                xt1 = xtp.tile([P, NKT // 2, QCW], MMD, tag="xt")
                if c == 0:
                    # interleave with the remaining wq pieces so the first
                    # Q-projection matmuls start as early as possible
                    nc.sync.dma_start(out=xt0[:, 0:4, :], in_=xt_view[0][:, 0:4, :])
                    nc.sync.dma_start(out=wq_sb[:, 4:16, :], in_=_wqv[:, 4:16, :])
                    nc.sync.dma_start(out=xt0[:, 4:8, :], in_=xt_view[0][:, 4:8, :])
                    nc.sync.dma_start(out=xt1, in_=xt_view[0][:, 8:16, :])
                else:
                    nc.sync.dma_start(out=xt0, in_=xt_view[c][:, 0:8, :])
                    nc.sync.dma_start(out=xt1, in_=xt_view[c][:, 8:16, :])
                xth = (xt0, xt1)
                q_sb = qch.tile([P, HPC, QCW], MMD, tag="q")
                for j in range(HPC):
                    qp = psA.tile([P, QCW], F32, tag="ps")
                    for k in range(NKT):
                        nc.tensor.matmul(
                            qp, wq_sb[:, k, j * DH:(j + 1) * DH],
                            xth[k // 8][:, k % 8, :],
                            start=(k == 0), stop=(k == NKT - 1))
                    nc.scalar.activation(q_sb[:, j, :], qp, AFT.Identity,
                                         bias=bq_sb[:, j, :], scale=SCALE)
                    kp = psA.tile([P, QCW], F32, tag="ps")
                    for k in range(NKT):
                        nc.tensor.matmul(
                            kp, wk_sb[:, k, j * DH:(j + 1) * DH],
                            xth[k // 8][:, k % 8, :],
                            start=(k == 0), stop=(k == NKT - 1))
                    nc.scalar.activation(k_sb[:, j, qc * QCW:(qc + 1) * QCW], kp,
                                         AFT.Identity, bias=bk_sb[:, j, :], scale=SCALE)
                for tt in range(KTPC):
                    vp = psA.tile([P, QCW], F32, tag="ps")
                    for k in range(NKT):
                        nc.tensor.matmul(
                            vp[:, 0:DSH],
                            xth[k // 8][:, k % 8, tt * P:(tt + 1) * P],
                            wv_sb[:, k, :],
                            start=(k == 0), stop=(k == NKT - 1))
                    nc.vector.tensor_copy(v_sb[:, qc * KTPC + tt, :], vp[:, 0:DSH])

                # ---- causal attention for q-chunk qc, both heads, with the
                # previous chunk's out-projection interleaved ----
                a_sb = ach.tile([P, HPC, QCW], MMD, tag="a")
                nkt_q = (qc + 1) * KTPC
                units = ([(tt, jc) for tt in range(KTPC) for jc in range(NJC)]
                         if prev is not None else [])
                ui = 0
                for h in range(HPC):
                    sacc = sacp.tile([P, QCW], F32, tag="sacc")
                    at = psT.tile([P, QCW], F32, tag="at")
                    for kt in range(nkt_q):
                        tdiag = kt - qc * KTPC
                        off = max(tdiag, 0) * P
                        w = QCW - off
                        sp = psS.tile([P, QCW], F32, tag="sc")
                        nc.tensor.matmul(
                            sp[:, 0:w], k_sb[:, h, kt * P:(kt + 1) * P],
                            q_sb[:, h, off:QCW], start=True, stop=True)
                        if tdiag >= 0:
                            nc.vector.tensor_add(sp[:, 0:P], sp[:, 0:P], tri_sb)
                        et = expp.tile([P, QCW], MMD, tag="exp")
                        nc.scalar.activation(et[:, 0:w], sp[:, 0:w], AFT.Exp)
                        if kt == 0:
                            nc.vector.tensor_copy(sacc, et)
                        else:
                            nc.vector.tensor_add(sacc[:, off:QCW], sacc[:, off:QCW],
                                                 et[:, 0:w])
                        nc.tensor.matmul(
                            at[:, off:QCW], v_sb[:, kt, h * DH:(h + 1) * DH],
                            et[:, 0:w], start=(kt == 0), stop=(kt == nkt_q - 1))
                        if ui < len(units):
                            outproj_unit(prev[0], prev[1], prev[2], *units[ui])
                            ui += 1
                    sacc_r = smlp.tile([P, QCW], MMD, tag="saccr", bufs=2)
                    nc.scalar.copy(sacc_r, sacc)
                    dnb = psD.tile([P, QCW], F32, tag="dn")
                    nc.tensor.matmul(dnb, ones_mat, sacc_r, start=True, stop=True)
                    rcf = smlp.tile([P, QCW], F32, tag="rcf", bufs=2)
                    nc.vector.reciprocal(rcf, dnb)
                    nc.vector.tensor_mul(a_sb[:, h, :], at, rcf)
                while ui < len(units):
                    outproj_unit(prev[0], prev[1], prev[2], *units[ui])
                    ui += 1
                prev = (b, qc, a_sb)

        # flush the final chunk's out-projection
        for tt in range(KTPC):
            for jc in range(NJC):
                outproj_unit(prev[0], prev[1], prev[2], tt, jc)


def _build():
    nc = bacc.Bacc("TRN2", target_bir_lowering=False, debug=False,
                   num_devices=N_CORES)
    t = {
        "xt": nc.dram_tensor("xt", [D, TOK], BF16, kind="ExternalInput").ap(),
        "wqt": nc.dram_tensor("wqt", [D, DSH], BF16, kind="ExternalInput").ap(),
        "wkt": nc.dram_tensor("wkt", [D, DSH], BF16, kind="ExternalInput").ap(),
        "wvt": nc.dram_tensor("wvt", [D, DSH], BF16, kind="ExternalInput").ap(),
        "wot": nc.dram_tensor("wot", [DSH, D], BF16, kind="ExternalInput").ap(),
        "bqs": nc.dram_tensor("bqs", [HPC, P, 1], F32, kind="ExternalInput").ap(),
        "bks": nc.dram_tensor("bks", [HPC, P, 1], F32, kind="ExternalInput").ap(),
        "tri": nc.dram_tensor("tri", [P, P], F32, kind="ExternalInput").ap(),
        "onesc": nc.dram_tensor("onesc", [P, P], BF16, kind="ExternalInput").ap(),
        "outp": nc.dram_tensor("outp", [TOK, D], F32, kind="ExternalOutput").ap(),
    }
    with tile.TileContext(nc) as tc:
        _emit(tc, t)
    nc.compile()
    return nc


def _program():
    global _nc
    if _nc is None:
        _nc = _build()
    return _nc


def kernel(X, Wq, bq, Wk, bk, Wv, bv, Wo, bo):
    X = np.asarray(X, np.float32)
    Wq = np.asarray(Wq, np.float32)
    Wk = np.asarray(Wk, np.float32)
    Wv = np.asarray(Wv, np.float32)
    Wo = np.asarray(Wo, np.float32)
    bq = np.asarray(bq, np.float32)
    bk = np.asarray(bk, np.float32)
    bv = np.asarray(bv, np.float32)
    bo = np.asarray(bo, np.float32)

    nc = _program()

    XT = np.ascontiguousarray(X.reshape(TOK, D).T).astype(ml_dtypes.bfloat16)
    tri = np.where(np.arange(P)[:, None] <= np.arange(P)[None, :],
                   np.float32(0.0), np.float32(MASK_NEG)).astype(np.float32)
    ones_col = np.ones((P, P), ml_dtypes.bfloat16)

    in_maps = []
    for c in range(N_CORES):
        J = slice(c * DSH, (c + 1) * DSH)
        in_maps.append({
            "xt": XT,
            "wqt": np.ascontiguousarray(Wq[J, :].T).astype(ml_dtypes.bfloat16),
            "wkt": np.ascontiguousarray(Wk[J, :].T).astype(ml_dtypes.bfloat16),
            "wvt": np.ascontiguousarray(Wv[J, :].T).astype(ml_dtypes.bfloat16),
            "wot": np.ascontiguousarray(Wo[:, J].T).astype(ml_dtypes.bfloat16),
            "bqs": (bq[J] * SCALE).reshape(HPC, P, 1).astype(np.float32),
            "bks": (bk[J] * SCALE).reshape(HPC, P, 1).astype(np.float32),
            "tri": tri,
            "onesc": ones_col,
        })

    res = run_bass_kernel_spmd(nc, in_maps, list(range(N_CORES)), trace=TRACE)
    LAST["res"] = res

    out = res.results[0]["outp"].copy()
    for c in range(1, N_CORES):
        out += res.results[c]["outp"]
    out += (bo + Wo @ bv)[None, :].astype(np.float32)
    return out.reshape(B, S, D).astype(np.float32)
